# revision 1
# baseline (speedup 1.0000x reference)
"""Canny filter (nn_CannyFilter) Trainium2 Bass kernel.

Math: the reference pipeline collapses to
    s   = mean_c img                       (channel mean; done by DMA accumulate)
    b   = gauss3x3 (*) s                   (separable: [g0,g1,g0] x [g0,g1,g0])
    gx  = sobelx (*) b = [.5,1,.5]_col (x) [-1,0,1]_row
    gy  = sobely (*) b = [-1,0,1]_col (x) [.5,1,.5]_row
    gm  = sqrt(gx^2 + gy^2)
    t1  = ring (*) gm                      (ring = 3x3 ones minus center)
    out = ring (*) t1
(each conv zero-padded by 1; the 8 directional one-hot kernels sum to `ring`,
and the NMS conv over the 8 directions is the same `ring` again).

Layout: each 512x512 plane is ONE SBUF tile [128, 2048]: partition p holds
image rows 4p..4p+3 (each row = 512 contiguous floats). W-direction convs are
free-dim shifts (DVE); H-direction convs are shifted-diagonal matmuls on the
PE (fp32r), with no partition-halo problems since the whole plane is in-tile.

Sharding: pure data parallel, 4 images per core across 8 cores.
"""
import numpy as np
from contextlib import ExitStack

import concourse.bass as bass
import concourse.tile as tile
from concourse import bacc, mybir
from concourse.bass_utils import run_bass_kernel_spmd

N_CORES = 8
B_TOTAL = 32
B_PER = B_TOTAL // N_CORES  # 4 images per core
C, H, W = 3, 512, 512
P = 128          # SBUF partitions
RPP = H // P     # 4 rows per partition
FW = RPP * W     # 2048 free dim per plane

F32 = mybir.dt.float32
F32R = mybir.dt.float32r
AF = mybir.ActivationFunctionType
OP = mybir.AluOpType

# gaussian separable vector (mu=0, sigma=1, k=3 -> exactly separable)
_g1 = np.linspace(-1.0, 1.0, 3)
_gg = np.exp(-(_g1[None, :] ** 2 + _g1[:, None] ** 2) / 2.0) / (2.0 * np.pi)
_gg = _gg / _gg.sum()
_u, _s, _vt = np.linalg.svd(_gg)
_gv = np.abs(_u[:, 0]) * np.sqrt(_s[0])
G0, G1 = float(_gv[0]), float(_gv[1])


def _plane_view(dram_ap):
    """[H, W] dram AP -> [128, 2048] (partition p = rows 4p..4p+3)."""
    return dram_ap.rearrange("(p q) w -> p (q w)", q=RPP)


def _build_nc():
    nc = bacc.Bacc("TRN2", target_bir_lowering=False, debug=False,
                   num_devices=N_CORES)
    img_d = nc.dram_tensor("img", [B_PER, C, H, W], F32, kind="ExternalInput")
    out_d = nc.dram_tensor("out", [B_PER, H, W], F32, kind="ExternalOutput")

    with tile.TileContext(nc, pool_alloc_mode="queue") as tc, ExitStack() as ctx:
        cpool = ctx.enter_context(tc.tile_pool(name="consts", bufs=1))
        spool = ctx.enter_context(tc.tile_pool(name="splanes", bufs=3))
        bpool = ctx.enter_context(tc.tile_pool(name="bpl", bufs=3))
        ppool = ctx.enter_context(tc.tile_pool(name="planes", bufs=1))
        ppool2 = ctx.enter_context(tc.tile_pool(name="planes2", bufs=2))
        upool = ctx.enter_context(tc.tile_pool(name="utmp", bufs=1))
        opool = ctx.enter_context(tc.tile_pool(name="outs", bufs=2))
        psum = ctx.enter_context(tc.tile_pool(name="ps", bufs=4, space="PSUM"))

        # ---- shifted-diagonal lhsT constants ------------------------------
        # D[p, f] = f - p  (int32), then lhsT = (D == delta) * coef  (fp32r)
        dmat = cpool.tile([P, P], mybir.dt.int32)
        nc.gpsimd.iota(dmat[:], pattern=[[1, P]], base=0, channel_multiplier=-1)

        lhs_cache = {}

        def lhs(delta, coef):
            key = (delta, round(coef, 10))
            if key not in lhs_cache:
                t = cpool.tile([P, P], F32R, tag=f"lhs_{len(lhs_cache)}")
                nc.vector.tensor_scalar(t[:], dmat[:], float(delta), float(coef),
                                        OP.is_equal, OP.mult)
                lhs_cache[key] = t
            return lhs_cache[key]

        # H-direction conv as matmuls.  out row r=4p+c taps row r+dr:
        #   cc = c+dr in 0..3  -> same partition, block cc, diag delta=0
        #   cc = -1            -> partition p-1, block 3, lhsT delta=+1
        #   cc = 4             -> partition p+1, block 0, lhsT delta=-1
        def col_mm(ps_t, inputs, c0=0, c1=RPP):
            """inputs: list of (tile, taps); accumulate all H-conv taps into ps_t.
            Writes blocks c0..c1-1 into ps_t at local offsets."""
            for c in range(c0, c1):
                todo = []
                for x_t, taps in inputs:
                    xv = x_t[:]
                    for dr, coef in taps:
                        cc = c + dr
                        if cc == -1:
                            todo.append((lhs(+1, coef), xv, 3))
                        elif cc == RPP:
                            todo.append((lhs(-1, coef), xv, 0))
                        else:
                            todo.append((lhs(0, coef), xv, cc))
                for j, (lt, xv, sb) in enumerate(todo):
                    lc = c - c0
                    nc.tensor.matmul(
                        ps_t[:, lc * W:(lc + 1) * W], lt[:],
                        xv[:, sb * W:(sb + 1) * W],
                        start=(j == 0), stop=(j == len(todo) - 1))


        def v3(t):
            return t[:].rearrange("p (c w) -> p c w", w=W)

        # W-direction symmetric neighbor sum u[w] = x[w-1] + x[w+1]
        def row_u(x_t, engine, tag):
            u = upool.tile([P, FW], F32, tag=tag)
            uv, xv = v3(u), v3(x_t)
            engine.tensor_add(uv[:, :, 1:W - 1], xv[:, :, 0:W - 2], xv[:, :, 2:W])
            engine.tensor_copy(uv[:, :, 0:1], xv[:, :, 1:2])
            engine.tensor_copy(uv[:, :, W - 1:W], xv[:, :, W - 2:W - 1])
            return u

        s_tiles = []
        for _si in range(B_PER):
            s_i = spool.tile([P, FW], F32, tag="s")
            s_tiles.append(s_i)
        for ch in range(C):
            for i in range(B_PER):
                nc.gpsimd.dma_start(
                    s_tiles[i][:], _plane_view(img_d[i, ch]),
                    accum_op=(OP.bypass if ch == 0 else OP.add))

        from contextlib import nullcontext
        for i in range(B_PER):
          with (tc.high_priority(offset=i * 180) if i else nullcontext()):
            s = s_tiles[i]

            # ---- gauss: r1 = Grow(s)/g0 ; b = Gcol(g0*r1)/3 ---------------
            u = row_u(s, nc.vector, "ug")
            r1 = ppool.tile([P, FW], F32R, tag="r1")
            nc.vector.scalar_tensor_tensor(r1[:], s[:], G1 / G0, u[:],
                                           OP.mult, OP.add)
            m = G0 * G0 / 3.0
            gtaps = [(r1, [(-1, m), (0, G1 * G0 / 3.0), (1, m)])]
            b = bpool.tile([P, FW], F32R, tag="b")
            for h in range(2):
                ps_b = psum.tile([P, FW // 2], F32, tag="ps")
                col_mm(ps_b, gtaps, c0=2 * h, c1=2 * h + 2)
                nc.scalar.activation(b[:, h * (FW // 2):(h + 1) * (FW // 2)],
                                     ps_b[:], AF.Copy)

            # ---- sobel x: d = b[w+1]-b[w-1] ; gx = Acol(d) ----------------
            d = ppool.tile([P, FW], F32R, tag="d")
            dv, bv = v3(d), v3(b)
            nc.vector.tensor_sub(dv[:, :, 1:W - 1], bv[:, :, 2:W], bv[:, :, 0:W - 2])
            nc.vector.tensor_copy(dv[:, :, 0:1], bv[:, :, 1:2])
            nc.vector.tensor_scalar_mul(dv[:, :, W - 1:W], bv[:, :, W - 2:W - 1], -1.0)
            q1 = ppool.tile([P, FW], F32, tag="q1")
            for h in range(2):
                ps_gx = psum.tile([P, FW // 2], F32, tag="ps")
                col_mm(ps_gx, [(d, [(-1, 0.5), (0, 1.0), (1, 0.5)])], 2 * h, 2 * h + 2)
                nc.scalar.activation(q1[:, h * (FW // 2):(h + 1) * (FW // 2)],
                                     ps_gx[:], AF.Square)

            # ---- sobel y: a = 2b + u2 (=Arow(b)/0.5); gy = 0.5*Dcol(a) ----
            u2 = row_u(b, nc.gpsimd, "ua")
            a = ppool.tile([P, FW], F32R, tag="a")
            nc.vector.scalar_tensor_tensor(a[:], b[:], 2.0, u2[:], OP.mult, OP.add)
            q2 = ppool.tile([P, FW], F32, tag="q2")
            for h in range(2):
                ps_gy = psum.tile([P, FW // 2], F32, tag="ps")
                col_mm(ps_gy, [(a, [(-1, -0.5), (1, 0.5)])], 2 * h, 2 * h + 2)
                nc.scalar.activation(q2[:, h * (FW // 2):(h + 1) * (FW // 2)],
                                     ps_gy[:], AF.Square)

            # ---- gm = sqrt(q1 + q2) --------------------------------------
            nc.vector.tensor_add(q1[:], q1[:], q2[:])
            gm = ppool2.tile([P, FW], F32R, tag="gm")
            nc.scalar.activation(gm[:], q1[:], AF.Sqrt)

            # ---- ring 1: t1 = Bcol(Brow(gm)) - gm ------------------------
            u3 = row_u(gm, nc.gpsimd, "ub1")
            br = ppool.tile([P, FW], F32R, tag="br")
            nc.vector.tensor_add(br[:], u3[:], gm[:])
            t1 = ppool2.tile([P, FW], F32R, tag="t1")
            for h in range(2):
                ps_t1 = psum.tile([P, FW // 2], F32, tag="ps")
                col_mm(ps_t1, [(br, [(-1, 1.0), (0, 1.0), (1, 1.0)])], 2 * h, 2 * h + 2)
                sl = slice(h * (FW // 2), (h + 1) * (FW // 2))
                nc.vector.tensor_sub(t1[:, sl], ps_t1[:], gm[:, sl])

            # ---- ring 2: out = Bcol(Brow(t1)) - t1 -----------------------
            u4 = row_u(t1, nc.gpsimd, "ub2")
            br2 = ppool.tile([P, FW], F32R, tag="br2")
            nc.vector.tensor_add(br2[:], u4[:], t1[:])
            o = opool.tile([P, FW], F32, tag="o")
            for h in range(2):
                ps_o = psum.tile([P, FW // 2], F32, tag="ps")
                col_mm(ps_o, [(br2, [(-1, 1.0), (0, 1.0), (1, 1.0)])], 2 * h, 2 * h + 2)
                sl = slice(h * (FW // 2), (h + 1) * (FW // 2))
                nc.vector.tensor_sub(o[:, sl], ps_o[:], t1[:, sl])

            nc.sync.dma_start(_plane_view(out_d[i]), o[:])

    nc.compile()
    return nc


_NC = None


def _get_nc():
    global _NC
    if _NC is None:
        _NC = _build_nc()
    return _NC


def kernel(**inputs):
    img = np.ascontiguousarray(np.asarray(inputs["img"], dtype=np.float32))
    nc = _get_nc()
    in_maps = [{"img": img[B_PER * c:B_PER * (c + 1)]} for c in range(N_CORES)]
    res = run_bass_kernel_spmd(nc, in_maps, list(range(N_CORES)))
    out = np.concatenate([res.results[c]["out"] for c in range(N_CORES)], axis=0)
    return out[:, None, :, :]



# revision 4
# speedup vs baseline: 2.7361x; 2.7361x over previous
"""Canny filter (nn_CannyFilter) Trainium2 Bass kernel — v3.

Math (per plane s = sum_c img_c; global scale factors folded into the
matmul coefficients):
    g  = Gr(sb)/G1            row gauss (shared by gx and gy paths)
    r1 = Dr(g)  = DrGr(s)/G1      gx = (G1/3)*(AcGc)col(r1)
    r2 = Ar(g)  = ArGr(s)/G1      gy = (G1/3)*(DcGc)col(r2)
    gm = sqrt(gx^2 + gy^2)
    u = Br(gm);  y = Bc(u)                       [= B(gm)]
    v = Br(y);   out = Bc(v) + gm - 2y           [= ring(ring(gm))]
where the "+gm - 2y" terms ride the ring-B matmul accumulation as extra
diag-0 taps, and every column conv is per-block 512-col matmuls with
merged shifted-diagonal lhsT (one matmul per distinct input block).

Layout: plane tiles are [128, 4, 516]: partition p holds rows 4p..4p+3,
each row zero-padded by 2 cols per side (W data in cols 2..514), so all
row (W-direction) convs are plain shifted 2-input adds with no edge
fixups.  Intermediates bf16 (DVE 2x tensor-tensor / 4x tensor-scalar
perf modes); psum and final output fp32.

Engines: DVE row convs + glue; PE column convs; Act squares/sqrt/psum
evacuation; Pool only issues the accumulate DMAs (SWDGE); SP issues the
rest of the DMAs (HWDGE).  Sharding: data parallel, 4 images per core.
"""
import numpy as np
from contextlib import ExitStack

import concourse.bass as bass
import concourse.tile as tile
from concourse import bacc, mybir
from concourse.bass_utils import run_bass_kernel_spmd

N_CORES = 8
B_TOTAL = 32
B_PER = B_TOTAL // N_CORES      # 4 images per core
C, H, W = 3, 512, 512
P = 128                         # SBUF partitions
RPP = H // P                    # 4 rows per partition
WP = W + 4                      # padded row (2 zero cols each side)
FWP = RPP * WP                  # free elems per padded plane

F32 = mybir.dt.float32
BF16 = mybir.dt.bfloat16
AF = mybir.ActivationFunctionType
OP = mybir.AluOpType

# gaussian separable vector (mu=0, sigma=1, k=3 -> exactly separable)
_a = float(np.exp(-0.5))
_nrm = 2.0 * _a + 1.0
G0, G1 = _a / _nrm, 1.0 / _nrm
E2, E1, E0 = G0 / 2.0, G0 + G1 / 2.0, G0 + G1   # Ac*Gc composite taps

_AL = G0 / G1        # row-gauss fold: g_true = _AL * (p + sb/_AL)
GX_TAPS = {d: c * _AL for d, c in
           {-2: E2 * G1 / 3, -1: E1 * G1 / 3, 0: E0 * G1 / 3,
            1: E1 * G1 / 3, 2: E2 * G1 / 3}.items()}
GY_TAPS = {d: c * _AL / 2 for d, c in
           {-2: -G0 * G1 / 3, -1: -G1 * G1 / 3,
            1: G1 * G1 / 3, 2: G0 * G1 / 3}.items()}
ONES = {-1: 1.0, 0: 1.0, 1: 1.0}


def _dram_plane(dram_ap):
    """[H, W] dram AP -> [128, 4, 512] (partition p = rows 4p..4p+3)."""
    return dram_ap.rearrange("(p q) w -> p q w", q=RPP)


def _build_nc():
    nc = bacc.Bacc("TRN2", target_bir_lowering=False, debug=False,
                   num_devices=N_CORES)
    img_d = nc.dram_tensor("img", [B_PER, C, H, W], F32, kind="ExternalInput")
    out_d = nc.dram_tensor("out", [B_PER, H, W], F32, kind="ExternalOutput")

    with tile.TileContext(nc, pool_alloc_mode="queue") as tc, ExitStack() as ctx:
        cpool = ctx.enter_context(tc.tile_pool(name="consts", bufs=1))
        sp = ctx.enter_context(tc.tile_pool(name="s", bufs=2))
        bp = ctx.enter_context(tc.tile_pool(name="bplanes", bufs=2))
        op_ = ctx.enter_context(tc.tile_pool(name="outs", bufs=2))
        psum = ctx.enter_context(tc.tile_pool(name="ps", bufs=2, space="PSUM"))

        def pv(t):
            return t[:].rearrange("p (q w) -> p q w", w=WP)

        def psv(t):
            return t[:].rearrange("p (q w) -> p q w", w=W)

        # ---- shifted-diagonal lhsT constants ------------------------------
        # D[p, f] = f - p (int32); merged lhsT = sum_i (D == delta_i)*coef_i
        dmat = cpool.tile([P, P], mybir.dt.int32)
        nc.gpsimd.iota(dmat[:], pattern=[[1, P]], base=0, channel_multiplier=-1)
        # S[p, f] = p + f: (S == 0) is the single entry [0, 0]; (S == 254)
        # is [127, 127].  Used for the composite-vs-sequential zero-padding
        # boundary corrections at image rows 0 and 511.
        smat = cpool.tile([P, P], mybir.dt.int32)
        nc.gpsimd.iota(smat[:], pattern=[[1, P]], base=0, channel_multiplier=1)
        lhs_cache = {}

        def lhs(diags):
            """diags: tuple of (delta, coef) merged into one bf16 lhsT."""
            key = tuple(sorted((d, round(c, 10)) for d, c in diags))
            if key in lhs_cache:
                return lhs_cache[key]
            acc = cpool.tile([P, P], F32, tag="lacc", name="lacc")
            d0, c0 = diags[0]
            nc.vector.tensor_scalar(acc[:], dmat[:], float(d0), float(c0),
                                    OP.is_equal, OP.mult)
            for d, c in diags[1:]:
                tmp = cpool.tile([P, P], F32, tag="ltmp", name="ltmp")
                nc.vector.tensor_scalar(tmp[:], dmat[:], float(d), float(c),
                                        OP.is_equal, OP.mult)
                nc.vector.tensor_add(acc[:], acc[:], tmp[:])
            t = cpool.tile([P, P], BF16, tag=f"lhs_{len(lhs_cache)}",
                           name="lhs")
            nc.vector.tensor_copy(t[:], acc[:])
            lhs_cache[key] = t
            return t

        edge_cache = {}

        def edge_lhs(sval, coef):
            key = (sval, round(coef, 12))
            if key in edge_cache:
                return edge_cache[key]
            acc = cpool.tile([P, P], F32, tag="lacc", name="elacc")
            nc.vector.tensor_scalar(acc[:], smat[:], float(sval), float(coef),
                                    OP.is_equal, OP.mult)
            t = cpool.tile([P, P], BF16, tag=f"elhs_{len(edge_cache)}",
                           name="elhs")
            nc.vector.tensor_copy(t[:], acc[:])
            edge_cache[key] = t
            return t

        def col_conv_block(ps_ap, c, inputs, edge=None):
            """H-direction conv of output block c into a [128, 512] psum
            region.  inputs: list of (padded [128,4,516] view, {dr: coef}).
            One matmul per (source, distinct input block), diagonals merged
            into a single lhsT."""
            items = []
            for src_v, taps in inputs:
                by_cb = {}
                for dr, coef in taps.items():
                    cc = c + dr
                    s = cc // RPP          # floor div: -1//4 == -1
                    cb = cc - RPP * s
                    by_cb.setdefault(cb, []).append((-s, coef))
                for cb, diags in sorted(by_cb.items()):
                    items.append((src_v, cb, lhs(tuple(diags))))
            if edge is not None:
                # (src_v, coef_top, coef_bot): exact fix for the phantom
                # virtual-row path kept by a fused 3x3*3x3 column composite
                src_v, ct, cb_ = edge
                if c == 0:
                    items.append((src_v, 0, edge_lhs(0, ct)))
                elif c == RPP - 1:
                    items.append((src_v, RPP - 1, edge_lhs(2 * P - 2, cb_)))
            for j, (src_v, cb, lt) in enumerate(items):
                nc.tensor.matmul(
                    ps_ap, lt[:], src_v[:, cb, 2:2 + W],
                    start=(j == 0), stop=(j == len(items) - 1))

        def col_conv(ps_t, half, inputs, edge=None):
            for lc, c in enumerate(range(2 * half, 2 * half + 2)):
                col_conv_block(ps_t[:, lc * W:(lc + 1) * W], c, inputs, edge)

        def bplane(tag, bufs=None):
            return bp.tile([P, FWP], BF16, tag=tag, name=tag, bufs=bufs)

        def pad_zero(t, i=None, bufs=2):
            # pads live in the physical buffer; after the ring wraps they are
            # already zero (interior ops never touch them), so skip the memset
            if i is not None and i >= bufs:
                return
            v = pv(t)
            nc.vector.memset(v[:, :, 0:2], 0.0)
            nc.vector.memset(v[:, :, WP - 2:WP], 0.0)

        ii = slice(2, 2 + W)       # interior W columns
        im = slice(1, 1 + W)       # shifted left by 1
        ip = slice(3, 3 + W)       # shifted right by 1

        TT = nc.vector.tensor_tensor

        # -- PE p-state warm-up ---------------------------------------------
        # A serial chain of cheap f32r matmuls issued first: during the
        # DMA-fill phase they keep the tensor engine continuously busy, so
        # pe_busy_start stays pinned and real matmuls run at full clock.
        # They rotate the gxy psum ring, so the first real gx matmul simply
        # queues behind the last warm-up (~fill-length by construction).
        N_WARM = 80
        wlhs = cpool.tile([P, P], F32, tag="wlhs", name="wlhs")
        nc.vector.tensor_scalar(wlhs[:], dmat[:], 0.0, 0.0,
                                OP.is_equal, OP.mult)
        for _w in range(N_WARM):
            ps_w = psum.tile([P, 2 * W], F32, tag="gxy", name="psw")
            nc.tensor.matmul(ps_w[:, 0:P], wlhs[:], wlhs[:],
                             start=True, stop=True)

        # -- loads ----------------------------------------------------------
        # s   <- ch0 (HWDGE) then += ch1 (SWDGE accumulate, the only SWDGE
        #        issue per image so the Pool queue stays shallow)
        # s2  <- ch2 (HWDGE, interleaved with ch0 so image i's planes finish
        #        before image i+1's start)
        # sb = bf16(s + s2) later fuses the final channel add.
        s_tiles, s2_tiles = {}, {}
        for i in range(B_PER):
            s_i = sp.tile([P, FWP], F32, tag="s", name="s")
            s_tiles[i] = s_i
            sv = pv(s_i)
            # all loads on the one SWDGE queue: issue order = device order,
            # so image i's three planes finish before image i+1's start.
            # ch2 (independent tile) sits between ch0 and the ch1-accumulate
            # to hide the WAW semaphore gap on the s tile.
            nc.gpsimd.dma_start(sv[:, :, ii], _dram_plane(img_d[i, 0]))
            s2_i = sp.tile([P, RPP * W], F32, tag="s2", name="s2")
            s2_tiles[i] = s2_i
            nc.gpsimd.dma_start(
                s2_i[:].rearrange("p (q w) -> p q w", w=W),
                _dram_plane(img_d[i, 2]))
            nc.gpsimd.dma_start(sv[:, :, ii], _dram_plane(img_d[i, 1]),
                                accum_op=OP.add)

        def prio(stage, i):
            # stage-major priorities: later images' early stages outrank
            # earlier images' late stages (allocation order is unchanged).
            tc.cur_priority = 1000 + stage * 10000 + i * 1000

        for i in range(B_PER):
            # -- sb = bf16(s + s2); g = Gr(sb)/G1 ---------------------------
            prio(0, i)
            sb = bplane("sb")
            pad_zero(sb, i, 2)
            TT(pv(sb)[:, :, ii], pv(s_tiles[i])[:, :, ii],
               s2_tiles[i][:].rearrange("p (q w) -> p q w", w=W), OP.add)
            sbv = pv(sb)
            p = bplane("p")
            TT(pv(p)[:, :, ii], sbv[:, :, im], sbv[:, :, ip], OP.add)
            sbs = bplane("sbs")
            nc.vector.tensor_scalar_mul(pv(sbs)[:, :, ii], sbv[:, :, ii],
                                        1.0 / _AL)
            g = bplane("g")
            pad_zero(g, i, 2)
            TT(pv(g)[:, :, ii], pv(p)[:, :, ii], pv(sbs)[:, :, ii], OP.add)
            gv = pv(g)

            # -- r1 = Dr(g); r2 = Ar(g) -------------------------------------
            prio(1, i)
            r1 = bplane("r1")
            TT(pv(r1)[:, :, ii], gv[:, :, ip], gv[:, :, im], OP.subtract)
            q = bplane("q")
            TT(pv(q)[:, :, ii], gv[:, :, im], gv[:, :, ip], OP.add)
            g2 = bplane("g2")
            nc.vector.tensor_scalar_mul(pv(g2)[:, :, ii], gv[:, :, ii], 2.0)
            r2 = bplane("r2")
            TT(pv(r2)[:, :, ii], pv(q)[:, :, ii], pv(g2)[:, :, ii], OP.add)

            # -- cols: gx -> q1 = gx^2 ; gy -> q2 = gy^2 --------------------
            prio(2, i)
            q1 = bplane("q1")
            for h in range(2):
                ps_t = psum.tile([P, 2 * W], F32, tag="gxy", name="ps")
                col_conv(ps_t, h, [(pv(r1), GX_TAPS)],
                         edge=(pv(r1), -0.5 * G0 * G1 / 3 * _AL,
                               -0.5 * G0 * G1 / 3 * _AL))
                nc.scalar.activation(pv(q1)[:, 2 * h:2 * h + 2, ii],
                                     psv(ps_t), AF.Square)
            q2 = bplane("q2")
            for h in range(2):
                ps_t = psum.tile([P, 2 * W], F32, tag="gxy", name="ps")
                col_conv(ps_t, h, [(pv(r2), GY_TAPS)],
                         edge=(pv(r2), G0 * G1 / 3 * _AL / 2,
                               -G0 * G1 / 3 * _AL / 2))
                nc.scalar.activation(pv(q2)[:, 2 * h:2 * h + 2, ii],
                                     psv(ps_t), AF.Square)

            # -- gm = sqrt(q1 + q2) -----------------------------------------
            prio(3, i)
            q12 = bplane("q12")
            TT(pv(q12)[:, :, ii], pv(q1)[:, :, ii], pv(q2)[:, :, ii], OP.add)
            gm = bplane("gm", bufs=3)
            pad_zero(gm, i, 3)
            nc.scalar.activation(pv(gm)[:, :, ii], pv(q12)[:, :, ii], AF.Sqrt)
            gmv = pv(gm)

            # -- ring A: u = Br(gm); y = Bc(u) ------------------------------
            prio(4, i)
            ua = bplane("ua")
            TT(pv(ua)[:, :, ii], gmv[:, :, im], gmv[:, :, ip], OP.add)
            u = bplane("u")
            TT(pv(u)[:, :, ii], pv(ua)[:, :, ii], gmv[:, :, ii], OP.add)
            y = bplane("y", bufs=3)
            pad_zero(y, i, 3)
            for b in range(RPP):
                ps_t = psum.tile([P, W], F32, tag="ringA", name="ps")
                col_conv_block(ps_t[:], b, [(pv(u), ONES)])
                nc.scalar.activation(
                    pv(y)[:, b:b + 1, ii],
                    ps_t[:].rearrange("p (q w) -> p q w", w=W), AF.Copy)
            yv = pv(y)

            # -- ring B: v = Br(y); out = Bc(v) + gm - 2y -------------------
            prio(5, i)
            va = bplane("va")
            TT(pv(va)[:, :, ii], yv[:, :, im], yv[:, :, ip], OP.add)
            v = bplane("v")
            TT(pv(v)[:, :, ii], pv(va)[:, :, ii], yv[:, :, ii], OP.add)
            o = op_.tile([P, FWP], F32, tag="o", name="o")
            for b in range(RPP):
                ps_t = psum.tile([P, W], F32, tag="ringB", name="ps")
                col_conv_block(ps_t[:], b, [(pv(v), ONES),
                                            (gmv, {0: 1.0}),
                                            (yv, {0: -2.0})])
                nc.scalar.activation(
                    pv(o)[:, b:b + 1, ii],
                    ps_t[:].rearrange("p (q w) -> p q w", w=W), AF.Copy)
            for h in range(2):
                nc.sync.dma_start(
                    _dram_plane(out_d[i])[:, 2 * h:2 * h + 2, :],
                    pv(o)[:, 2 * h:2 * h + 2, ii])

    nc.compile()
    return nc


_NC = None


def _get_nc():
    global _NC
    if _NC is None:
        _NC = _build_nc()
    return _NC


def kernel(**inputs):
    img = np.ascontiguousarray(np.asarray(inputs["img"], dtype=np.float32))
    nc = _get_nc()
    in_maps = [{"img": img[B_PER * c:B_PER * (c + 1)]} for c in range(N_CORES)]
    res = run_bass_kernel_spmd(nc, in_maps, list(range(N_CORES)))
    out = np.concatenate([res.results[c]["out"] for c in range(N_CORES)], axis=0)
    return out[:, None, :, :]


# revision 5
# speedup vs baseline: 2.8108x; 1.0273x over previous
"""Canny filter (nn_CannyFilter) Trainium2 Bass kernel — v3.

Math (per plane s = sum_c img_c; global scale factors folded into the
matmul coefficients):
    g  = Gr(sb)/G1            row gauss (shared by gx and gy paths)
    r1 = Dr(g)  = DrGr(s)/G1      gx = (G1/3)*(AcGc)col(r1)
    r2 = Ar(g)  = ArGr(s)/G1      gy = (G1/3)*(DcGc)col(r2)
    gm = sqrt(gx^2 + gy^2)
    u = Br(gm);  y = Bc(u)                       [= B(gm)]
    v = Br(y);   out = Bc(v) + gm - 2y           [= ring(ring(gm))]
where the "+gm - 2y" terms ride the ring-B matmul accumulation as extra
diag-0 taps, and every column conv is per-block 512-col matmuls with
merged shifted-diagonal lhsT (one matmul per distinct input block).

Layout: plane tiles are [128, 4, 516]: partition p holds rows 4p..4p+3,
each row zero-padded by 2 cols per side (W data in cols 2..514), so all
row (W-direction) convs are plain shifted 2-input adds with no edge
fixups.  Intermediates bf16 (DVE 2x tensor-tensor / 4x tensor-scalar
perf modes); psum and final output fp32.

Engines: DVE row convs + glue; PE column convs; Act squares/sqrt/psum
evacuation; Pool only issues the accumulate DMAs (SWDGE); SP issues the
rest of the DMAs (HWDGE).  Sharding: data parallel, 4 images per core.
"""
import numpy as np
from contextlib import ExitStack

import concourse.bass as bass
import concourse.tile as tile
from concourse import bacc, mybir
from concourse.bass_utils import run_bass_kernel_spmd

N_CORES = 8
B_TOTAL = 32
B_PER = B_TOTAL // N_CORES      # 4 images per core
C, H, W = 3, 512, 512
P = 128                         # SBUF partitions
RPP = H // P                    # 4 rows per partition
WP = W + 4                      # padded row (2 zero cols each side)
FWP = RPP * WP                  # free elems per padded plane

F32 = mybir.dt.float32
BF16 = mybir.dt.bfloat16
AF = mybir.ActivationFunctionType
OP = mybir.AluOpType

# gaussian separable vector (mu=0, sigma=1, k=3 -> exactly separable)
_a = float(np.exp(-0.5))
_nrm = 2.0 * _a + 1.0
G0, G1 = _a / _nrm, 1.0 / _nrm
E2, E1, E0 = G0 / 2.0, G0 + G1 / 2.0, G0 + G1   # Ac*Gc composite taps

_AL = G0 / G1        # row-gauss fold: g_true = _AL * (p + sb/_AL)
GX_TAPS = {d: c * _AL for d, c in
           {-2: E2 * G1 / 3, -1: E1 * G1 / 3, 0: E0 * G1 / 3,
            1: E1 * G1 / 3, 2: E2 * G1 / 3}.items()}
GY_TAPS = {d: c * _AL / 2 for d, c in
           {-2: -G0 * G1 / 3, -1: -G1 * G1 / 3,
            1: G1 * G1 / 3, 2: G0 * G1 / 3}.items()}
ONES = {-1: 1.0, 0: 1.0, 1: 1.0}


def _dram_plane(dram_ap):
    """[H, W] dram AP -> [128, 4, 512] (partition p = rows 4p..4p+3)."""
    return dram_ap.rearrange("(p q) w -> p q w", q=RPP)


def _build_nc():
    nc = bacc.Bacc("TRN2", target_bir_lowering=False, debug=False,
                   num_devices=N_CORES)
    img_d = nc.dram_tensor("img", [B_PER, C, H, W], F32, kind="ExternalInput")
    out_d = nc.dram_tensor("out", [B_PER, H, W], F32, kind="ExternalOutput")

    with tile.TileContext(nc, pool_alloc_mode="queue") as tc, ExitStack() as ctx:
        cpool = ctx.enter_context(tc.tile_pool(name="consts", bufs=1))
        sp = ctx.enter_context(tc.tile_pool(name="s", bufs=2))
        bp = ctx.enter_context(tc.tile_pool(name="bplanes", bufs=2))
        op_ = ctx.enter_context(tc.tile_pool(name="outs", bufs=2))
        psum = ctx.enter_context(tc.tile_pool(name="ps", bufs=2, space="PSUM"))

        def pv(t):
            return t[:].rearrange("p (q w) -> p q w", w=WP)

        def psv(t):
            return t[:].rearrange("p (q w) -> p q w", w=W)

        # ---- shifted-diagonal lhsT constants ------------------------------
        # D[p, f] = f - p (int32); merged lhsT = sum_i (D == delta_i)*coef_i
        dmat = cpool.tile([P, P], mybir.dt.int32)
        nc.gpsimd.iota(dmat[:], pattern=[[1, P]], base=0, channel_multiplier=-1)
        # S[p, f] = p + f: (S == 0) is the single entry [0, 0]; (S == 254)
        # is [127, 127].  Used for the composite-vs-sequential zero-padding
        # boundary corrections at image rows 0 and 511.
        smat = cpool.tile([P, P], mybir.dt.int32)
        nc.gpsimd.iota(smat[:], pattern=[[1, P]], base=0, channel_multiplier=1)
        lhs_cache = {}

        def lhs(diags, sterm=None):
            """diags: tuple of (delta, coef) merged into one bf16 lhsT.
            sterm: optional (sval, coef) single-entry term via smat."""
            key = (tuple(sorted((d, round(c, 10)) for d, c in diags)),
                   None if sterm is None else (sterm[0], round(sterm[1], 12)))
            if key in lhs_cache:
                return lhs_cache[key]
            acc = cpool.tile([P, P], F32, tag="lacc", name="lacc")
            d0, c0 = diags[0]
            nc.vector.tensor_scalar(acc[:], dmat[:], float(d0), float(c0),
                                    OP.is_equal, OP.mult)
            terms = [(dmat, d, c) for d, c in diags[1:]]
            if sterm is not None:
                terms.append((smat, sterm[0], sterm[1]))
            for mat, d, c in terms:
                tmp = cpool.tile([P, P], F32, tag="ltmp", name="ltmp")
                nc.vector.tensor_scalar(tmp[:], mat[:], float(d), float(c),
                                        OP.is_equal, OP.mult)
                nc.vector.tensor_add(acc[:], acc[:], tmp[:])
            t = cpool.tile([P, P], BF16, tag=f"lhs_{len(lhs_cache)}",
                           name="lhs")
            nc.vector.tensor_copy(t[:], acc[:])
            lhs_cache[key] = t
            return t

        edge_cache = {}

        def edge_lhs(sval, coef):
            key = (sval, round(coef, 12))
            if key in edge_cache:
                return edge_cache[key]
            acc = cpool.tile([P, P], F32, tag="lacc", name="elacc")
            nc.vector.tensor_scalar(acc[:], smat[:], float(sval), float(coef),
                                    OP.is_equal, OP.mult)
            t = cpool.tile([P, P], BF16, tag=f"elhs_{len(edge_cache)}",
                           name="elhs")
            nc.vector.tensor_copy(t[:], acc[:])
            edge_cache[key] = t
            return t

        def col_conv_block(ps_ap, c, inputs, edge=None):
            """H-direction conv of output block c into a [128, 512] psum
            region.  inputs: list of (padded [128,4,516] view, {dr: coef}).
            One matmul per (source, distinct input block), diagonals merged
            into a single lhsT."""
            # edge: (src_v, coef_top, coef_bot) — exact fix for the phantom
            # virtual-row path kept by a fused 3x3*3x3 column composite.
            # Merged into an existing same-block lhsT when one exists.
            esrc = ecb = esterm = None
            if edge is not None and c == 0:
                esrc, ecb, esterm = edge[0], 0, (0, edge[1])
            elif edge is not None and c == RPP - 1:
                esrc, ecb, esterm = edge[0], RPP - 1, (2 * P - 2, edge[2])
            items = []
            for src_v, taps in inputs:
                by_cb = {}
                for dr, coef in taps.items():
                    cc = c + dr
                    s = cc // RPP          # floor div: -1//4 == -1
                    cb = cc - RPP * s
                    by_cb.setdefault(cb, []).append((-s, coef))
                for cb, diags in sorted(by_cb.items()):
                    if esrc is src_v and cb == ecb:
                        items.append((src_v, cb, lhs(tuple(diags), esterm)))
                        esrc = None
                    else:
                        items.append((src_v, cb, lhs(tuple(diags))))
            if esrc is not None:
                items.append((esrc, ecb, edge_lhs(*esterm)))
            for j, (src_v, cb, lt) in enumerate(items):
                nc.tensor.matmul(
                    ps_ap, lt[:], src_v[:, cb, 2:2 + W],
                    start=(j == 0), stop=(j == len(items) - 1))

        def col_conv(ps_t, half, inputs, edge=None):
            for lc, c in enumerate(range(2 * half, 2 * half + 2)):
                col_conv_block(ps_t[:, lc * W:(lc + 1) * W], c, inputs, edge)

        def bplane(tag, bufs=None):
            return bp.tile([P, FWP], BF16, tag=tag, name=tag, bufs=bufs)

        def pad_zero(t, i=None, bufs=2):
            # pads live in the physical buffer; after the ring wraps they are
            # already zero (interior ops never touch them), so skip the memset
            if i is not None and i >= bufs:
                return
            v = pv(t)
            nc.vector.memset(v[:, :, 0:2], 0.0)
            nc.vector.memset(v[:, :, WP - 2:WP], 0.0)

        ii = slice(2, 2 + W)       # interior W columns
        im = slice(1, 1 + W)       # shifted left by 1
        ip = slice(3, 3 + W)       # shifted right by 1

        TT = nc.vector.tensor_tensor

        # -- PE p-state warm-up ---------------------------------------------
        # A serial chain of cheap f32r matmuls issued first: during the
        # DMA-fill phase they keep the tensor engine continuously busy, so
        # pe_busy_start stays pinned and real matmuls run at full clock.
        # They rotate the gxy psum ring, so the first real gx matmul simply
        # queues behind the last warm-up (~fill-length by construction).
        N_WARM = 140
        wlhs = cpool.tile([P, P], F32, tag="wlhs", name="wlhs")
        nc.vector.tensor_scalar(wlhs[:], dmat[:], 0.0, 0.0,
                                OP.is_equal, OP.mult)
        for _w in range(N_WARM):
            ps_w = psum.tile([P, 2 * W], F32, tag="gxy", name="psw")
            nc.tensor.matmul(ps_w[:, 0:64], wlhs[:], wlhs[:, 0:64],
                             start=True, stop=True)

        # -- loads ----------------------------------------------------------
        # s   <- ch0 (HWDGE) then += ch1 (SWDGE accumulate, the only SWDGE
        #        issue per image so the Pool queue stays shallow)
        # s2  <- ch2 (HWDGE, interleaved with ch0 so image i's planes finish
        #        before image i+1's start)
        # sb = bf16(s + s2) later fuses the final channel add.
        s_tiles, s2_tiles = {}, {}
        for i in range(B_PER):
            s_i = sp.tile([P, FWP], F32, tag="s", name="s")
            s_tiles[i] = s_i
            sv = pv(s_i)
            # all loads on the one SWDGE queue: issue order = device order,
            # so image i's three planes finish before image i+1's start.
            # ch2 (independent tile) sits between ch0 and the ch1-accumulate
            # to hide the WAW semaphore gap on the s tile.
            nc.gpsimd.dma_start(sv[:, :, ii], _dram_plane(img_d[i, 0]))
            s2_i = sp.tile([P, RPP * W], F32, tag="s2", name="s2")
            s2_tiles[i] = s2_i
            nc.gpsimd.dma_start(
                s2_i[:].rearrange("p (q w) -> p q w", w=W),
                _dram_plane(img_d[i, 2]))
            nc.gpsimd.dma_start(sv[:, :, ii], _dram_plane(img_d[i, 1]),
                                accum_op=OP.add)

        def prio(stage, i):
            # stage-major priorities: later images' early stages outrank
            # earlier images' late stages (allocation order is unchanged).
            tc.cur_priority = 1000 + stage * 10000 + i * 30000

        for i in range(B_PER):
            # -- sb = bf16(s + s2); g = Gr(sb)/G1 ---------------------------
            prio(0, i)
            sb = bplane("sb")
            pad_zero(sb, i, 2)
            TT(pv(sb)[:, :, ii], pv(s_tiles[i])[:, :, ii],
               s2_tiles[i][:].rearrange("p (q w) -> p q w", w=W), OP.add)
            sbv = pv(sb)
            p = bplane("p")
            TT(pv(p)[:, :, ii], sbv[:, :, im], sbv[:, :, ip], OP.add)
            sbs = bplane("sbs")
            nc.vector.tensor_scalar_mul(pv(sbs)[:, :, ii], sbv[:, :, ii],
                                        1.0 / _AL)
            g = bplane("g")
            pad_zero(g, i, 2)
            TT(pv(g)[:, :, ii], pv(p)[:, :, ii], pv(sbs)[:, :, ii], OP.add)
            gv = pv(g)

            # -- r1 = Dr(g); r2 = Ar(g) -------------------------------------
            prio(1, i)
            r1 = bplane("r1")
            TT(pv(r1)[:, :, ii], gv[:, :, ip], gv[:, :, im], OP.subtract)
            q = bplane("q")
            TT(pv(q)[:, :, ii], gv[:, :, im], gv[:, :, ip], OP.add)
            g2 = bplane("g2")
            nc.vector.tensor_scalar_mul(pv(g2)[:, :, ii], gv[:, :, ii], 2.0)
            r2 = bplane("r2")
            TT(pv(r2)[:, :, ii], pv(q)[:, :, ii], pv(g2)[:, :, ii], OP.add)

            # -- cols: gx -> q1 = gx^2 ; gy -> q2 = gy^2 --------------------
            prio(2, i)
            q1 = bplane("q1")
            for h in range(2):
                ps_t = psum.tile([P, 2 * W], F32, tag="gxy", name="ps")
                col_conv(ps_t, h, [(pv(r1), GX_TAPS)],
                         edge=(pv(r1), -0.5 * G0 * G1 / 3 * _AL,
                               -0.5 * G0 * G1 / 3 * _AL))
                nc.scalar.activation(pv(q1)[:, 2 * h:2 * h + 2, ii],
                                     psv(ps_t), AF.Square)
            q2 = bplane("q2")
            for h in range(2):
                ps_t = psum.tile([P, 2 * W], F32, tag="gxy", name="ps")
                col_conv(ps_t, h, [(pv(r2), GY_TAPS)],
                         edge=(pv(r2), G0 * G1 / 3 * _AL / 2,
                               -G0 * G1 / 3 * _AL / 2))
                nc.scalar.activation(pv(q2)[:, 2 * h:2 * h + 2, ii],
                                     psv(ps_t), AF.Square)

            # -- gm = sqrt(q1 + q2) -----------------------------------------
            prio(3, i)
            q12 = bplane("q12")
            TT(pv(q12)[:, :, ii], pv(q1)[:, :, ii], pv(q2)[:, :, ii], OP.add)
            gm = bplane("gm", bufs=3)
            pad_zero(gm, i, 3)
            nc.scalar.activation(pv(gm)[:, :, ii], pv(q12)[:, :, ii], AF.Sqrt)
            gmv = pv(gm)

            # -- ring A: u = Br(gm); y = Bc(u) ------------------------------
            prio(4, i)
            ua = bplane("ua")
            TT(pv(ua)[:, :, ii], gmv[:, :, im], gmv[:, :, ip], OP.add)
            u = bplane("u")
            TT(pv(u)[:, :, ii], pv(ua)[:, :, ii], gmv[:, :, ii], OP.add)
            y = bplane("y", bufs=3)
            pad_zero(y, i, 3)
            for b in range(RPP):
                ps_t = psum.tile([P, W], F32, tag="ringA", name="ps")
                col_conv_block(ps_t[:], b, [(pv(u), ONES)])
                nc.scalar.activation(
                    pv(y)[:, b:b + 1, ii],
                    ps_t[:].rearrange("p (q w) -> p q w", w=W), AF.Copy)
            yv = pv(y)

            # -- ring B: v = Br(y); out = Bc(v) + gm - 2y -------------------
            prio(5, i)
            va = bplane("va")
            TT(pv(va)[:, :, ii], yv[:, :, im], yv[:, :, ip], OP.add)
            v = bplane("v")
            TT(pv(v)[:, :, ii], pv(va)[:, :, ii], yv[:, :, ii], OP.add)
            o = op_.tile([P, FWP], F32, tag="o", name="o")
            for b in range(RPP):
                ps_t = psum.tile([P, W], F32, tag="ringB", name="ps")
                col_conv_block(ps_t[:], b, [(pv(v), ONES),
                                            (gmv, {0: 1.0}),
                                            (yv, {0: -2.0})])
                nc.scalar.activation(
                    pv(o)[:, b:b + 1, ii],
                    ps_t[:].rearrange("p (q w) -> p q w", w=W), AF.Copy)
            for h in range(2):
                nc.sync.dma_start(
                    _dram_plane(out_d[i])[:, 2 * h:2 * h + 2, :],
                    pv(o)[:, 2 * h:2 * h + 2, ii])

    nc.compile()
    return nc


_NC = None


def _get_nc():
    global _NC
    if _NC is None:
        _NC = _build_nc()
    return _NC


def kernel(**inputs):
    img = np.ascontiguousarray(np.asarray(inputs["img"], dtype=np.float32))
    nc = _get_nc()
    in_maps = [{"img": img[B_PER * c:B_PER * (c + 1)]} for c in range(N_CORES)]
    res = run_bass_kernel_spmd(nc, in_maps, list(range(N_CORES)))
    out = np.concatenate([res.results[c]["out"] for c in range(N_CORES)], axis=0)
    return out[:, None, :, :]


# revision 7
# speedup vs baseline: 2.9000x; 1.0318x over previous
"""Canny filter (nn_CannyFilter) Trainium2 Bass kernel — v3.

Math (per plane s = sum_c img_c; global scale factors folded into the
matmul coefficients):
    g  = Gr(sb)/G1            row gauss (shared by gx and gy paths)
    r1 = Dr(g)  = DrGr(s)/G1      gx = (G1/3)*(AcGc)col(r1)
    r2 = Ar(g)  = ArGr(s)/G1      gy = (G1/3)*(DcGc)col(r2)
    gm = sqrt(gx^2 + gy^2)
    u = Br(gm);  y = Bc(u)                       [= B(gm)]
    v = Br(y);   out = Bc(v) + gm - 2y           [= ring(ring(gm))]
where the "+gm - 2y" terms ride the ring-B matmul accumulation as extra
diag-0 taps, and every column conv is per-block 512-col matmuls with
merged shifted-diagonal lhsT (one matmul per distinct input block).

Layout: plane tiles are [128, 4, 516]: partition p holds rows 4p..4p+3,
each row zero-padded by 2 cols per side (W data in cols 2..514), so all
row (W-direction) convs are plain shifted 2-input adds with no edge
fixups.  Intermediates bf16 (DVE 2x tensor-tensor / 4x tensor-scalar
perf modes); psum and final output fp32.

Engines: DVE row convs + glue; PE column convs; Act squares/sqrt/psum
evacuation; Pool only issues the accumulate DMAs (SWDGE); SP issues the
rest of the DMAs (HWDGE).  Sharding: data parallel, 4 images per core.
"""
import numpy as np
from contextlib import ExitStack

import concourse.bass as bass
import concourse.tile as tile
from concourse import bacc, mybir
from concourse.bass_utils import run_bass_kernel_spmd

N_CORES = 8
B_TOTAL = 32
B_PER = B_TOTAL // N_CORES      # 4 images per core
C, H, W = 3, 512, 512
P = 128                         # SBUF partitions
RPP = H // P                    # 4 rows per partition
WP = W + 4                      # padded row (2 zero cols each side)
FWP = RPP * WP                  # free elems per padded plane

F32 = mybir.dt.float32
BF16 = mybir.dt.bfloat16
AF = mybir.ActivationFunctionType
OP = mybir.AluOpType

# gaussian separable vector (mu=0, sigma=1, k=3 -> exactly separable)
_a = float(np.exp(-0.5))
_nrm = 2.0 * _a + 1.0
G0, G1 = _a / _nrm, 1.0 / _nrm
E2, E1, E0 = G0 / 2.0, G0 + G1 / 2.0, G0 + G1   # Ac*Gc composite taps

_AL = G0 / G1        # row-gauss fold: g_true = _AL * (p + sb/_AL)
GX_TAPS = {d: c * _AL for d, c in
           {-2: E2 * G1 / 3, -1: E1 * G1 / 3, 0: E0 * G1 / 3,
            1: E1 * G1 / 3, 2: E2 * G1 / 3}.items()}
GY_TAPS = {d: c * _AL / 2 for d, c in
           {-2: -G0 * G1 / 3, -1: -G1 * G1 / 3,
            1: G1 * G1 / 3, 2: G0 * G1 / 3}.items()}
ONES = {-1: 1.0, 0: 1.0, 1: 1.0}


def _dram_plane(dram_ap):
    """[H, W] dram AP -> [128, 4, 512] (partition p = rows 4p..4p+3)."""
    return dram_ap.rearrange("(p q) w -> p q w", q=RPP)


def _build_nc():
    nc = bacc.Bacc("TRN2", target_bir_lowering=False, debug=False,
                   num_devices=N_CORES)
    img_d = nc.dram_tensor("img", [B_PER, C, H, W], F32, kind="ExternalInput")
    out_d = nc.dram_tensor("out", [B_PER, H, W], F32, kind="ExternalOutput")

    with tile.TileContext(nc, pool_alloc_mode="queue") as tc, ExitStack() as ctx:
        cpool = ctx.enter_context(tc.tile_pool(name="consts", bufs=1))
        sp = ctx.enter_context(tc.tile_pool(name="s", bufs=2))
        bp = ctx.enter_context(tc.tile_pool(name="bplanes", bufs=2))
        op_ = ctx.enter_context(tc.tile_pool(name="outs", bufs=2))
        psum = ctx.enter_context(tc.tile_pool(name="ps", bufs=2, space="PSUM"))

        def pv(t):
            return t[:].rearrange("p (q w) -> p q w", w=WP)

        def psv(t):
            return t[:].rearrange("p (q w) -> p q w", w=W)

        # ---- shifted-diagonal lhsT constants ------------------------------
        # D[p, f] = f - p (int32); merged lhsT = sum_i (D == delta_i)*coef_i
        dmat = cpool.tile([P, P], mybir.dt.int32)
        nc.gpsimd.iota(dmat[:], pattern=[[1, P]], base=0, channel_multiplier=-1)
        # S[p, f] = p + f: (S == 0) is the single entry [0, 0]; (S == 254)
        # is [127, 127].  Used for the composite-vs-sequential zero-padding
        # boundary corrections at image rows 0 and 511.
        smat = cpool.tile([P, P], mybir.dt.int32)
        nc.gpsimd.iota(smat[:], pattern=[[1, P]], base=0, channel_multiplier=1)
        lhs_cache = {}

        def lhs(diags, sterm=None):
            """diags: tuple of (delta, coef) merged into one bf16 lhsT.
            sterm: optional (sval, coef) single-entry term via smat."""
            key = (tuple(sorted((d, round(c, 10)) for d, c in diags)),
                   None if sterm is None else (sterm[0], round(sterm[1], 12)))
            if key in lhs_cache:
                return lhs_cache[key]
            acc = cpool.tile([P, P], F32, tag="lacc", name="lacc")
            d0, c0 = diags[0]
            nc.vector.tensor_scalar(acc[:], dmat[:], float(d0), float(c0),
                                    OP.is_equal, OP.mult)
            terms = [(dmat, d, c) for d, c in diags[1:]]
            if sterm is not None:
                terms.append((smat, sterm[0], sterm[1]))
            for mat, d, c in terms:
                tmp = cpool.tile([P, P], F32, tag="ltmp", name="ltmp")
                nc.vector.tensor_scalar(tmp[:], mat[:], float(d), float(c),
                                        OP.is_equal, OP.mult)
                nc.vector.tensor_add(acc[:], acc[:], tmp[:])
            t = cpool.tile([P, P], BF16, tag=f"lhs_{len(lhs_cache)}",
                           name="lhs")
            nc.vector.tensor_copy(t[:], acc[:])
            lhs_cache[key] = t
            return t

        edge_cache = {}

        def edge_lhs(sval, coef):
            key = (sval, round(coef, 12))
            if key in edge_cache:
                return edge_cache[key]
            acc = cpool.tile([P, P], F32, tag="lacc", name="elacc")
            nc.vector.tensor_scalar(acc[:], smat[:], float(sval), float(coef),
                                    OP.is_equal, OP.mult)
            t = cpool.tile([P, P], BF16, tag=f"elhs_{len(edge_cache)}",
                           name="elhs")
            nc.vector.tensor_copy(t[:], acc[:])
            edge_cache[key] = t
            return t

        def col_conv_block(ps_ap, c, inputs, edge=None):
            """H-direction conv of output block c into a [128, 512] psum
            region.  inputs: list of (padded [128,4,516] view, {dr: coef}).
            One matmul per (source, distinct input block), diagonals merged
            into a single lhsT."""
            # edge: (src_v, coef_top, coef_bot) — exact fix for the phantom
            # virtual-row path kept by a fused 3x3*3x3 column composite.
            # Merged into an existing same-block lhsT when one exists.
            esrc = ecb = esterm = None
            if edge is not None and c == 0:
                esrc, ecb, esterm = edge[0], 0, (0, edge[1])
            elif edge is not None and c == RPP - 1:
                esrc, ecb, esterm = edge[0], RPP - 1, (2 * P - 2, edge[2])
            items = []
            for src_v, taps in inputs:
                by_cb = {}
                for dr, coef in taps.items():
                    cc = c + dr
                    s = cc // RPP          # floor div: -1//4 == -1
                    cb = cc - RPP * s
                    by_cb.setdefault(cb, []).append((-s, coef))
                for cb, diags in sorted(by_cb.items()):
                    if esrc is src_v and cb == ecb:
                        items.append((src_v, cb, lhs(tuple(diags), esterm)))
                        esrc = None
                    else:
                        items.append((src_v, cb, lhs(tuple(diags))))
            if esrc is not None:
                items.append((esrc, ecb, edge_lhs(*esterm)))
            for j, (src_v, cb, lt) in enumerate(items):
                nc.tensor.matmul(
                    ps_ap, lt[:], src_v[:, cb, 2:2 + W],
                    start=(j == 0), stop=(j == len(items) - 1))

        def col_conv(ps_t, half, inputs, edge=None):
            for lc, c in enumerate(range(2 * half, 2 * half + 2)):
                col_conv_block(ps_t[:, lc * W:(lc + 1) * W], c, inputs, edge)

        def bplane(tag, bufs=None):
            return bp.tile([P, FWP], BF16, tag=tag, name=tag, bufs=bufs)

        def pad_zero(t, i=None, bufs=2):
            # pads live in the physical buffer; after the ring wraps they are
            # already zero (interior ops never touch them), so skip the memset
            if i is not None and i >= bufs:
                return
            v = pv(t)
            nc.vector.memset(v[:, :, 0:2], 0.0)
            nc.vector.memset(v[:, :, WP - 2:WP], 0.0)

        ii = slice(2, 2 + W)       # interior W columns
        im = slice(1, 1 + W)       # shifted left by 1
        ip = slice(3, 3 + W)       # shifted right by 1

        TT = nc.vector.tensor_tensor

        # -- PE p-state warm-up ---------------------------------------------
        # A serial chain of cheap f32r matmuls issued first: during the
        # DMA-fill phase they keep the tensor engine continuously busy, so
        # pe_busy_start stays pinned and real matmuls run at full clock.
        # They rotate the gxy psum ring, so the first real gx matmul simply
        # queues behind the last warm-up (~fill-length by construction).
        N_WARM = 140
        wlhs = cpool.tile([P, P], F32, tag="wlhs", name="wlhs")
        nc.vector.tensor_scalar(wlhs[:], dmat[:], 0.0, 0.0,
                                OP.is_equal, OP.mult)
        for _w in range(N_WARM):
            ps_w = psum.tile([P, 2 * W], F32, tag="gxy", name="psw")
            nc.tensor.matmul(ps_w[:, 0:32], wlhs[:], wlhs[:, 0:32],
                             start=True, stop=True)

        # -- loads ----------------------------------------------------------
        # s   <- ch0 (HWDGE) then += ch1 (SWDGE accumulate, the only SWDGE
        #        issue per image so the Pool queue stays shallow)
        # s2  <- ch2 (HWDGE, interleaved with ch0 so image i's planes finish
        #        before image i+1's start)
        # sb = bf16(s + s2) later fuses the final channel add.
        s_tiles, s2_tiles = {}, {}
        for i in range(B_PER):
            s_i = sp.tile([P, FWP], F32, tag="s", name="s")
            s_tiles[i] = s_i
            sv = pv(s_i)
            # all loads on the one SWDGE queue: issue order = device order,
            # so image i's three planes finish before image i+1's start.
            # ch2 (independent tile) sits between ch0 and the ch1-accumulate
            # to hide the WAW semaphore gap on the s tile.
            nc.gpsimd.dma_start(sv[:, :, ii], _dram_plane(img_d[i, 0]))
            s2_i = sp.tile([P, RPP * W], F32, tag="s2", name="s2")
            s2_tiles[i] = s2_i
            nc.gpsimd.dma_start(
                s2_i[:].rearrange("p (q w) -> p q w", w=W),
                _dram_plane(img_d[i, 2]))
            nc.gpsimd.dma_start(sv[:, :, ii], _dram_plane(img_d[i, 1]),
                                accum_op=OP.add)

        def prio(stage, i):
            # stage-major priorities: later images' early stages outrank
            # earlier images' late stages (allocation order is unchanged).
            tc.cur_priority = 1000 + stage * 12000 + i * 30000

        for i in range(B_PER):
            # -- sb = bf16(s + s2); g = Gr(sb)/G1 ---------------------------
            prio(0, i)
            sb = bplane("sb")
            pad_zero(sb, i, 2)
            TT(pv(sb)[:, :, ii], pv(s_tiles[i])[:, :, ii],
               s2_tiles[i][:].rearrange("p (q w) -> p q w", w=W), OP.add)
            sbv = pv(sb)
            p = bplane("p")
            TT(pv(p)[:, :, ii], sbv[:, :, im], sbv[:, :, ip], OP.add)
            sbs = bplane("sbs")
            nc.vector.tensor_scalar_mul(pv(sbs)[:, :, ii], sbv[:, :, ii],
                                        1.0 / _AL)
            g = bplane("g")
            pad_zero(g, i, 2)
            TT(pv(g)[:, :, ii], pv(p)[:, :, ii], pv(sbs)[:, :, ii], OP.add)
            gv = pv(g)

            # -- r1 = Dr(g); r2 = Ar(g) -------------------------------------
            r1 = bplane("r1")
            TT(pv(r1)[:, :, ii], gv[:, :, ip], gv[:, :, im], OP.subtract)
            q = bplane("q")
            TT(pv(q)[:, :, ii], gv[:, :, im], gv[:, :, ip], OP.add)
            g2 = bplane("g2")
            nc.vector.tensor_scalar_mul(pv(g2)[:, :, ii], gv[:, :, ii], 2.0)
            r2 = bplane("r2")
            TT(pv(r2)[:, :, ii], pv(q)[:, :, ii], pv(g2)[:, :, ii], OP.add)

            # -- cols: gx -> q1 = gx^2 ; gy -> q2 = gy^2 --------------------
            prio(2, i)
            q1 = bplane("q1")
            for h in range(2):
                ps_t = psum.tile([P, 2 * W], F32, tag="gxy", name="ps")
                col_conv(ps_t, h, [(pv(r1), GX_TAPS)],
                         edge=(pv(r1), -0.5 * G0 * G1 / 3 * _AL,
                               -0.5 * G0 * G1 / 3 * _AL))
                nc.scalar.activation(pv(q1)[:, 2 * h:2 * h + 2, ii],
                                     psv(ps_t), AF.Square)
            q2 = bplane("q2")
            for h in range(2):
                ps_t = psum.tile([P, 2 * W], F32, tag="gxy", name="ps")
                col_conv(ps_t, h, [(pv(r2), GY_TAPS)],
                         edge=(pv(r2), G0 * G1 / 3 * _AL / 2,
                               -G0 * G1 / 3 * _AL / 2))
                nc.scalar.activation(pv(q2)[:, 2 * h:2 * h + 2, ii],
                                     psv(ps_t), AF.Square)

            # -- gm = sqrt(q1 + q2): sum via two diag taps in psum ----------
            prio(3, i)
            gm = bplane("gm", bufs=3)
            pad_zero(gm, i, 3)
            for h in range(2):
                ps_t = psum.tile([P, 2 * W], F32, tag="gxy", name="ps")
                col_conv(ps_t, h, [(pv(q1), {0: 1.0}), (pv(q2), {0: 1.0})])
                nc.scalar.activation(pv(gm)[:, 2 * h:2 * h + 2, ii],
                                     psv(ps_t), AF.Sqrt)
            gmv = pv(gm)

            # -- ring A: u = Br(gm); y = Bc(u) ------------------------------
            prio(4, i)
            ua = bplane("ua")
            TT(pv(ua)[:, :, ii], gmv[:, :, im], gmv[:, :, ip], OP.add)
            u = bplane("u")
            TT(pv(u)[:, :, ii], pv(ua)[:, :, ii], gmv[:, :, ii], OP.add)
            y = bplane("y", bufs=3)
            pad_zero(y, i, 3)
            for b in range(RPP):
                ps_t = psum.tile([P, W], F32, tag="ringA", name="ps")
                col_conv_block(ps_t[:], b, [(pv(u), ONES)])
                nc.scalar.activation(
                    pv(y)[:, b:b + 1, ii],
                    ps_t[:].rearrange("p (q w) -> p q w", w=W), AF.Copy)
            yv = pv(y)

            # -- ring B: v = Br(y); out = Bc(v) + gm - 2y -------------------
            prio(5, i)
            va = bplane("va")
            TT(pv(va)[:, :, ii], yv[:, :, im], yv[:, :, ip], OP.add)
            v = bplane("v")
            TT(pv(v)[:, :, ii], pv(va)[:, :, ii], yv[:, :, ii], OP.add)
            o = op_.tile([P, FWP], F32, tag="o", name="o")
            for b in range(RPP):
                ps_t = psum.tile([P, W], F32, tag="ringB", name="ps")
                col_conv_block(ps_t[:], b, [(pv(v), ONES),
                                            (gmv, {0: 1.0}),
                                            (yv, {0: -2.0})])
                nc.scalar.activation(
                    pv(o)[:, b:b + 1, ii],
                    ps_t[:].rearrange("p (q w) -> p q w", w=W), AF.Copy)
            for h in range(2):
                nc.sync.dma_start(
                    _dram_plane(out_d[i])[:, 2 * h:2 * h + 2, :],
                    pv(o)[:, 2 * h:2 * h + 2, ii])

    nc.compile()
    return nc


_NC = None


def _get_nc():
    global _NC
    if _NC is None:
        _NC = _build_nc()
    return _NC


def kernel(**inputs):
    img = np.ascontiguousarray(np.asarray(inputs["img"], dtype=np.float32))
    nc = _get_nc()
    in_maps = [{"img": img[B_PER * c:B_PER * (c + 1)]} for c in range(N_CORES)]
    res = run_bass_kernel_spmd(nc, in_maps, list(range(N_CORES)))
    out = np.concatenate([res.results[c]["out"] for c in range(N_CORES)], axis=0)
    return out[:, None, :, :]
